# revision 20
# baseline (speedup 1.0000x reference)
"""CaptionLoss (LSTM decode + cross-entropy) on 8 Trainium2 NeuronCores.

Strategy (v2):
  - Host: build teacher-forced token ids, gather+transpose embedding rows,
    transpose weights into T-layout (feature on partition). All matmul
    operands quantized to fp8 e4m3 with x16 scaling.
  - Device (one SPMD program on 8 cores, no collectives):
      * gates: per step the PSUM accumulation group receives
        bias (K=1 bf16 matmul) + W_ih@x_t (fp8 DoubleRow) + W_hh@h_{t-1}
        (fp8 DoubleRow) -- no separate ih precompute, no gate adds.
        Tanh (ACT) reads gate PSUM directly; all-tanh formulation
        (sigmoid(x)=(1+tanh(x/2))/2, c-state stored as 2c) keeps every
        ACT func in one LUT table.
      * LSTM tail on DVE as tensor_scalar (4x mode) + tensor_tensor (2x
        mode) ops over a contiguous Z=[c2|tg|to] state tile; h written
        fp8 x16 into hsT columns.
      * vocab shard (4000 cols/core): fp8 DoubleRow matmuls + K=2 fp8 DR
        bias pass into PSUM; Pool (mostly) and DVE copy the PSUM tiles
        into a wide SBUF logit buffer; ONE 4000-wide ACT Exp with
        accum_out row-sum per 128-row chunk (amortizes ACT per-op
        overhead ~8x and keeps the Exp off the gate critical path).
      * emission interleaves vocab matmuls/copies/exps into the LSTM
        step slack; hs exported in chunks as steps complete.
  - Host: sum partial exp-sums across cores, target-logit dot from the
    exported hs, final log/sum reduction in f64.
"""

import numpy as np
import ml_dtypes as mld

B = 64
T = 50
TP1 = T + 1
R = TP1 * B          # 3264 sequence rows, t-major (r = t*B + b)
H = 512
E = 512
G = 4 * H            # 2048 gate rows
V = 32000
NC = 8
VS = V // NC         # 4000 vocab shard
START_IDX = 1
STOP_IDX = 2
KC = H // 128        # 4 contraction chunks
MC_G = G // 128      # 16 gate row chunks
MC_R = (R + 127) // 128   # 26 row chunks (last has 64 valid rows)
NT_FC = 8            # vocab shard split into 8 psum chunks (7x512 + 416)
NV = 512
SCL = 16.0           # fp8 operand scale; products carry 256x

_BUILT = None

import os
CFG_DBUDGET = int(os.environ.get("K_DBUDGET", "2"))  # D pair-units per step
CFG_PSX = int(os.environ.get("K_PSX", "2"))
CFG_PSC = int(os.environ.get("K_PSC", "2"))


def _build():
    import concourse.bacc as bacc
    import concourse.mybir as mybir
    import concourse.tile as tile

    f32 = mybir.dt.float32
    f8 = mybir.dt.float8e4
    bf16 = mybir.dt.bfloat16
    DR = mybir.MatmulPerfMode.DoubleRow
    AF = mybir.ActivationFunctionType
    from concourse.alu_op_type import AluOpType

    nc = bacc.Bacc("TRN2", target_bir_lowering=False, debug=False,
                   num_devices=NC)

    # ---- DRAM I/O (fp8 operands pre-scaled x16 by host) --------------
    xTb_d = nc.dram_tensor("xTb", [H, B], f8, kind="ExternalInput")
    xTf_d = nc.dram_tensor("xTf", [H, B], f32, kind="ExternalInput")
    XT_d = nc.dram_tensor("XT", [E, R], f8, kind="ExternalInput")
    WihT_d = nc.dram_tensor("WihT", [E, G], f8, kind="ExternalInput")
    WhhT_d = nc.dram_tensor("WhhT", [H, G], f8, kind="ExternalInput")
    biasT_d = nc.dram_tensor("biasT", [1, G], bf16, kind="ExternalInput")
    fcWT_d = nc.dram_tensor("fcWT", [H, VS], f8, kind="ExternalInput")
    fcb8_d = nc.dram_tensor("fcb8", [1, 2 * VS], f8, kind="ExternalInput")

    S_d = nc.dram_tensor("S", [128, MC_R], f32, kind="ExternalOutput")
    hs_d = nc.dram_tensor("hs", [128, KC * R], f8, kind="ExternalOutput")

    with tile.TileContext(nc) as tc:
        with (tc.tile_pool(name="glob", bufs=1) as gp,
              tc.tile_pool(name="xs", bufs=3) as xsp,
              tc.tile_pool(name="gs", bufs=2) as gsp,
              tc.tile_pool(name="psX", bufs=CFG_PSX, space="PSUM") as psX,
              tc.tile_pool(name="psC", bufs=CFG_PSC, space="PSUM") as psC):
            # ---- constants / state ----------------------------------
            WhhT = gp.tile([128, KC * G], f8)
            for pr in range(2):
                nc.sync.dma_start(
                    out=WhhT[:, :].rearrange("p (k g) -> p k g", k=KC)
                        [:, 2 * pr:2 * pr + 2, :],
                    in_=WhhT_d.ap().rearrange("(k p) g -> p k g", p=128)
                        [:, 2 * pr:2 * pr + 2, :])
            WihT = gp.tile([128, KC * G], f8)
            for pr in range(2):
                nc.sync.dma_start(
                    out=WihT[:, :].rearrange("p (k g) -> p k g", k=KC)
                        [:, 2 * pr:2 * pr + 2, :],
                    in_=WihT_d.ap().rearrange("(k p) g -> p k g", p=128)
                        [:, 2 * pr:2 * pr + 2, :])
            biasT = gp.tile([1, G], bf16)
            nc.sync.dma_start(out=biasT[:, :], in_=biasT_d[:, :])
            ones64 = gp.tile([1, B], bf16)
            nc.gpsimd.memset(ones64[:, :], 1.0)
            xTb = gp.tile([128, KC * B], f8)
            nc.sync.dma_start(
                out=xTb[:, :].rearrange("p (k b) -> p k b", k=KC),
                in_=xTb_d.ap().rearrange("(k p) b -> p k b", p=128))
            # Z = [c | tg | to]; c initialized to x
            Z = gp.tile([128, 768], f32)
            nc.sync.dma_start(
                out=Z[:, 0:256].rearrange("p (k b) -> p k b", k=KC),
                in_=xTf_d.ap().rearrange("(k p) b -> p k b", p=128))
            fcW = gp.tile([128, KC * VS], f8)
            fcb8 = gp.tile([1, 2 * VS], f8)
            hsb = gp.tile([1, 256], f8)
            nc.gpsimd.memset(hsb[:, 0:128], SCL)
            nc.gpsimd.memset(hsb[:, 128:256], 0.0)
            hsT = gp.tile([128, KC * R], f8)
            S_all = gp.tile([128, MC_R * 4], f32)
            nc.vector.memset(S_all[:, :], 0.0)

            Wih3 = WihT[:, :].rearrange("p (k g) -> p k g", k=KC)
            Whh3 = WhhT[:, :].rearrange("p (k g) -> p k g", k=KC)
            hs3 = hsT[:, :].rearrange("p (k r) -> p k r", k=KC)
            fcW3 = fcW[:, :].rearrange("p (k v) -> p k v", k=KC)
            xTb3 = xTb[:, :].rearrange("p (k b) -> p k b", k=KC)

            # ---- XT chunk staging -----------------------------------
            n_chunks = []
            c0 = 0
            while c0 < R:
                w = min(512, R - c0)
                n_chunks.append((c0, w))
                c0 += w
            xt_tiles = {}

            def emit_B_dma(j):
                c0, w = n_chunks[j]
                xt = xsp.tile([128, KC * 512], f8, tag="xt")
                nc.sync.dma_start(
                    out=xt[:, :].rearrange("p (k n) -> p k n", k=KC)
                        [:, :, 0:w],
                    in_=XT_d.ap().rearrange(
                        "(k p) n -> p k n", p=128)[:, :, c0:c0 + w])
                xt_tiles[j] = xt

            # ---- per-step gate group + tail -------------------------
            # ps1 cols = [tg (m8..11) | to (m12..15)]
            # ps0 cols = [tf (m4..7) | ti (m0..3)]
            def emit_C(t, pump_exp=None):
                j = t // 8
                xt3 = xt_tiles[j][:, :].rearrange("p (k n) -> p k n", k=KC)
                xcols = slice((t % 8) * B, (t % 8) * B + B)
                if t == 0:
                    rhs_h = xTb3
                    hcols = slice(0, B)
                else:
                    rhs_h = hs3
                    hcols = slice((t - 1) * B, t * B)
                ps1 = psC.tile([128, 512], f32, tag="ps1")
                ps0 = psC.tile([128, 512], f32, tag="ps0")
                for ps, ms in ((ps1, (8, 9, 10, 11, 12, 13, 14, 15)),
                               (ps0, (4, 5, 6, 7, 0, 1, 2, 3))):
                    for ci, m in enumerate(ms):
                        col = ci * B
                        gsl = slice(m * 128, (m + 1) * 128)
                        nc.tensor.matmul(
                            ps[:, col:col + B],
                            biasT[:, gsl], ones64[:, :],
                            start=True, stop=False)
                        for pr in range(2):
                            nc.tensor.matmul(
                                ps[:, col:col + B],
                                Wih3[:, 2 * pr:2 * pr + 2, gsl],
                                xt3[:, 2 * pr:2 * pr + 2, xcols],
                                start=False, stop=False, perf_mode=DR)
                        for pr in range(2):
                            nc.tensor.matmul(
                                ps[:, col:col + B],
                                Whh3[:, 2 * pr:2 * pr + 2, gsl],
                                rhs_h[:, 2 * pr:2 * pr + 2, hcols],
                                start=False, stop=(pr == 1),
                                perf_mode=DR)
                # s1 = tanh(gates/512) -> [tg | to] in Z
                nc.scalar.activation(out=Z[:, 256:768], in_=ps1[:, :],
                                     func=AF.Tanh, scale=1.0 / 512)
                to8 = gsp.tile([128, 256], f32, tag="to8")
                nc.vector.tensor_scalar(
                    out=to8[:, :], in0=Z[:, 512:768], scalar1=8.0,
                    scalar2=8.0, op0=AluOpType.mult, op1=AluOpType.add)
                s0 = gsp.tile([128, 512], f32, tag="s0")
                nc.scalar.activation(out=s0[:, :], in_=ps0[:, :],
                                     func=AF.Tanh, scale=1.0 / 512)
                # exp slot 1: covers ACT wait for the DVE c'-chain
                if pump_exp is not None:
                    pump_exp()
                # P = (1+s0)/2 = [sig(f) | sig(i)];  w = P * [c | tg];
                # c' = w0 + w1   (c' = sig(f)*c + sig(i)*tg)
                P = gsp.tile([128, 512], f32, tag="P")
                nc.vector.tensor_scalar(
                    out=P[:, :], in0=s0[:, :], scalar1=0.5,
                    scalar2=0.5, op0=AluOpType.mult, op1=AluOpType.add)
                w2 = gsp.tile([128, 512], f32, tag="w2")
                nc.vector.tensor_tensor(
                    out=w2[:, :], in0=P[:, :], in1=Z[:, 0:512],
                    op=AluOpType.mult)
                nc.vector.tensor_tensor(
                    out=Z[:, 0:256], in0=w2[:, 0:256], in1=w2[:, 256:512],
                    op=AluOpType.add)
                th = gsp.tile([128, 256], f32, tag="th")
                nc.scalar.activation(out=th[:, :], in_=Z[:, 0:256],
                                     func=AF.Tanh, scale=1.0)
                # exp slot 2: covers ACT wait for h + next step's gate mms
                if pump_exp is not None:
                    pump_exp()
                # h*16 = (8 + 8*to) * tanh(c), written x16 fp8
                nc.vector.tensor_tensor(
                    out=hs3[:, :, t * B:(t + 1) * B], in0=to8[:, :],
                    in1=th[:, :], op=AluOpType.mult)

            # ---- vocab shard: pair matmuls -> 1024-wide exp ---------
            # D unit = (m, pair): two 512-col accumulation groups into one
            # 2-bank PSUM tile; one Exp+accum over the 1024-col pair reads
            # PSUM directly (no staging).
            def emit_D_mm(m, pair):
                mw = min(128, R - m * 128)
                ps = psX.tile([128, 1024], f32, tag="fps")
                for sub in range(2):
                    n = pair * 2 + sub
                    nw = min(NV, VS - n * NV)
                    o0 = sub * NV
                    for pr in range(2):
                        nc.tensor.matmul(
                            ps[0:mw, o0:o0 + nw],
                            hs3[:, 2 * pr:2 * pr + 2,
                                m * 128:m * 128 + mw],
                            fcW3[:, 2 * pr:2 * pr + 2,
                                 n * NV:n * NV + nw],
                            start=(pr == 0), stop=False, perf_mode=DR)
                    nc.tensor.matmul(
                        ps[0:mw, o0:o0 + nw],
                        hsb[:, :].rearrange("p (c m) -> p c m", c=2)
                            [:, :, 0:mw],
                        fcb8[:, :].rearrange("p (c v) -> p c v", c=2)
                            [:, :, n * NV:n * NV + nw],
                        start=False, stop=True, perf_mode=DR)
                return (ps, m, pair, mw)

            eo = gp.tile([128, 1024], bf16)

            def emit_D_exp(pend):
                ps, m, pair, mw = pend
                cw = min(1024, VS - pair * 1024)
                nc.scalar.activation(
                    out=eo[0:mw, 0:cw], in_=ps[0:mw, 0:cw], func=AF.Exp,
                    scale=1.0 / 256,
                    accum_out=S_all[0:mw, m * 4 + pair:m * 4 + pair + 1])

            # ---- interleaved emission -------------------------------
            emit_B_dma(0)
            emit_B_dma(1)
            for k in range(KC):
                nc.sync.dma_start(
                    out=fcW[:, k * VS:(k + 1) * VS],
                    in_=fcWT_d[k * 128:(k + 1) * 128, :])
            nc.sync.dma_start(out=fcb8[:, :], in_=fcb8_d[:, :])

            d_queue = [(m, pair) for m in range(MC_R) for pair in range(4)]
            d_next = 0
            exp_queue = []       # (ps, m, pair, mw) emitted, exp pending

            def pump_D(t, budget):
                # new matmuls whose hs rows completed BEFORE this step
                # (lagged so they never wait on the in-flight h write and
                # can fill PE idle time during the gate-chain tail)
                nonlocal d_next
                m_ready = (t - 2) // 2 if t >= 2 else -1
                n_emit = 0
                while n_emit < budget and d_next < len(d_queue):
                    m, pair = d_queue[d_next]
                    if t <= TP1 - 1 and m > m_ready:
                        break
                    exp_queue.append(emit_D_mm(m, pair))
                    d_next += 1
                    n_emit += 1

            def pump_exp():
                # one exp per call; fills the ACT gap it is emitted into
                if exp_queue:
                    emit_D_exp(exp_queue.pop(0))

            hs_exported = 0

            def export_hs(upto):
                nonlocal hs_exported
                if upto > hs_exported:
                    c0, c1 = hs_exported, upto
                    nc.sync.dma_start(
                        out=hs_d.ap().rearrange("p (k r) -> p k r", k=KC)
                            [:, :, c0:c1],
                        in_=hs3[:, :, c0:c1])
                    hs_exported = upto

            for t in range(TP1):
                emit_C(t, pump_exp)
                if t % 8 == 0:
                    j = t // 8 + 2
                    if j < len(n_chunks):
                        emit_B_dma(j)
                if t % 8 == 5:
                    export_hs((t - 1) * B)
                pump_D(t, CFG_DBUDGET)
            export_hs(R)
            while d_next < len(d_queue) or exp_queue:
                pump_exp()
                pump_D(TP1 + 10, CFG_DBUDGET)
                pump_exp()
            S_fin = gp.tile([128, MC_R], f32)
            nc.vector.reduce_sum(
                out=S_fin[:, :],
                in_=S_all[:, :].rearrange("p (m n) -> p m n", n=4),
                axis=mybir.AxisListType.X)
            nc.sync.dma_start(out=S_d[:, :], in_=S_fin[:, :])

    nc.compile()
    return nc


def _get_built():
    global _BUILT
    if _BUILT is None:
        _BUILT = _build()
    return _BUILT


def _q8(a):
    return np.clip(a, -240.0, 240.0).astype(mld.float8_e4m3)


def prep_in_maps(x, labels, emb, W_ih, W_hh, b_ih, b_hh, fc_W, fc_b):
    lab = labels.astype(np.int64)
    inputs = np.concatenate(
        [np.full((B, 1), START_IDX, np.int64), lab], axis=1)      # [B, 51]
    targets = np.concatenate(
        [lab, np.full((B, 1), STOP_IDX, np.int64)], axis=1)       # [B, 51]
    idx = inputs.T.reshape(-1)      # [3264] t-major
    tgt = targets.T.reshape(-1)

    # unified tanh(x/512): g-gate rows (the tanh gate) carry half scale
    gsc = np.ones((G, 1), np.float32)
    gsc[2 * H:3 * H] = 2.0
    base = {
        "xTb": _q8(np.ascontiguousarray(x.T) * SCL),
        "xTf": np.ascontiguousarray(x.T).astype(np.float32),
        "XT": _q8(np.ascontiguousarray(emb[idx].T) * SCL),
        "WihT": _q8(np.ascontiguousarray((W_ih * gsc).T) * SCL),
        "WhhT": _q8(np.ascontiguousarray((W_hh * gsc).T) * SCL),
        "biasT": ((b_ih + b_hh) * gsc[:, 0] * 256.0)[None, :]
            .astype(mld.bfloat16),
    }
    in_maps = []
    for c in range(NC):
        sh = slice(c * VS, (c + 1) * VS)
        fcb8 = np.zeros((1, 2 * VS), mld.float8_e4m3)
        fcb8[0, :VS] = _q8(fc_b[sh] * SCL)
        in_maps.append(dict(
            base,
            fcWT=_q8(np.ascontiguousarray(fc_W[sh].T) * SCL),
            fcb8=fcb8))
    return in_maps, tgt


def combine(results, tgt, fc_W, fc_b):
    S_rows = np.zeros(R, np.float64)
    for c in range(NC):
        S_rows += np.asarray(
            results[c]["S"], np.float64).T.reshape(-1)[:R]
    hs0 = np.asarray(results[0]["hs"]).astype(np.float32) / SCL   # [128, 4*R]
    hs_rows = hs0.reshape(128, KC, R).transpose(2, 1, 0).reshape(R, H)
    Wt = fc_W[tgt].astype(mld.bfloat16).astype(np.float32)        # [3264, 512]
    tgt_dot = (hs_rows * Wt).sum(1, dtype=np.float32)
    nll = np.log(S_rows) - (tgt_dot.astype(np.float64) + fc_b[tgt])
    return np.float32(nll.sum() / B)


def kernel(x, labels, emb, W_ih, W_hh, b_ih, b_hh, fc_W, fc_b):
    from concourse.bass_utils import run_bass_kernel_spmd

    x = np.asarray(x, np.float32)
    emb = np.asarray(emb, np.float32)
    W_ih = np.asarray(W_ih, np.float32)
    W_hh = np.asarray(W_hh, np.float32)
    b_ih = np.asarray(b_ih, np.float32)
    b_hh = np.asarray(b_hh, np.float32)
    fc_W = np.asarray(fc_W, np.float32)
    fc_b = np.asarray(fc_b, np.float32)

    in_maps, tgt = prep_in_maps(x, np.asarray(labels), emb, W_ih, W_hh,
                                b_ih, b_hh, fc_W, fc_b)
    nc = _get_built()
    res = run_bass_kernel_spmd(nc, in_maps, core_ids=list(range(NC)))
    return combine(res.results, tgt, fc_W, fc_b)
